# revision 28
# baseline (speedup 1.0000x reference)
"""Trainium2 Bass kernel for nn_AdaptiveSample (per-pixel 5x5 sampled softmax
aggregation), distributed over 8 NeuronCores.

Sharding: data-parallel over (batch, H): core i handles batch i//4, rows
[60*(i%4), 60*(i%4)+60). Halo rows are read directly from the full input on
the host (full_io), so no device collectives are needed.

Device layout: partitions = (x-half, row) -> 2*64 = 128 partitions per core
(60 owned rows + 2+2 halo rows per x-half). Free dim = (channel, x) with a
column halo. dx taps become free-dim offsets; dy taps are handled by loading
dy-shifted copies of the inputs straight from DRAM (compute engines cannot
start at arbitrary partitions, DMA can read any DRAM rows). The weighted sum
runs on the VectorEngine in bf16 (2x mode); transcendentals on ScalarEngine.
Even/odd-dx copies keep bf16 operands 4-byte aligned for the 2x DVE mode.

sample_idx is read on the host at call time and the kernel is compiled for
the unique (dy, dx) taps with multiplicities folded into the exp bias
(exp(x + ln m) = m*exp(x)).

guide_weight is all-ones per the problem spec; this is verified at runtime
and a numpy fallback handles the general case.
"""

import os
import sys

for _p in ("/opt/trn_rl_repo", "/root/.axon_site/_ro/trn_rl_repo"):
    if os.path.isdir(_p) and _p not in sys.path:
        sys.path.insert(0, _p)

import numpy as np
import ml_dtypes

import concourse.bass as bass
import concourse.bacc as bacc
import concourse.mybir as mybir
from concourse.tile import TileContext
from concourse.bass_utils import run_bass_kernel_spmd
from concourse.masks import make_identity

BF16 = ml_dtypes.bfloat16

K_SIZE = 5
SAMPLE_NUM = 15
DEPTH_MAX = 192.0

B, C, H, W = 2, 32, 240, 320
NCORES = 8
RCH = H * B // NCORES          # 60 owned rows per core
ROWS = RCH + 4                 # 64 rows incl. dy halo
YEXT = ROWS + 4                # 68 DRAM rows (dy-shifted loads need +-2 more)
XH = W // 2                    # 160: x is split in half across partitions
XW = XH + 4                    # 164: x window incl. dx halo
XD = XW + 4                    # 168 DRAM cols (parity-shifted loads)
PW = W + 10                    # padded row width for host prep

_compiled = {}


def _unique_taps(sample_idx):
    """-> sorted tuple of ((dy, dx), mult), dy/dx in [-2, 2]."""
    from collections import Counter
    cnt = Counter()
    for p in np.asarray(sample_idx).tolist():
        cnt[(p // K_SIZE - 2, p % K_SIZE - 2)] += 1
    return tuple(sorted(cnt.items()))


def _tap_src(dx):
    """-> (parity, x-offset) for a 160-wide slice of a parity tile."""
    par = dx & 1
    return par, 2 + dx - par


def _variants(taps):
    """Distinct (dy, parity) variant list, in tap (dy-sorted) order."""
    seen = []
    for (dy, dx), _ in taps:
        v = (dy, dx & 1)
        if v not in seen:
            seen.append(v)
    return seen


def _build(taps):
    """Build the per-core Bass program for the given unique taps."""
    U = len(taps)
    f32 = mybir.dt.float32
    bf = mybir.dt.bfloat16
    Alu = mybir.AluOpType
    Act = mybir.ActivationFunctionType

    dys = sorted({dy for (dy, _), _ in taps})
    by_dy = {d: [(j, (dy, dx), m) for j, ((dy, dx), m) in enumerate(taps)
                 if dy == d] for d in dys}
    variants = _variants(taps)
    vidx = {v: i for i, v in enumerate(variants)}
    NV = len(variants)

    nc = bacc.Bacc()

    ordered = [(j, (dy, dx), m) for dy in dys
               for j, (dy, dx), m in by_dy[dy]]
    mults = sorted({m for _, _, m in ordered})

    # Per-tap fully-shifted weight inputs (one fat op per pipeline stage);
    # per-(dy,parity) variant feature images for the MAC (dx via slices).
    d_feat = nc.declare_dram_parameter("feat", [NV, 128, C, XW], bf,
                                       isOutput=False)
    d_nrm = nc.declare_dram_parameter("nrm", [128, U, 3, XH], bf,
                                      isOutput=False)
    d_vld = nc.declare_dram_parameter("vld", [128, U, XH], bf,
                                      isOutput=False)
    d_nre = nc.declare_dram_parameter("nre", [128, 3, XH], bf, isOutput=False)
    d_idm = nc.declare_dram_parameter("idm", [len(mults), 128, 128], bf,
                                      isOutput=False)
    d_out = nc.declare_dram_parameter("out", [4, 128, C, XH // 4], f32,
                                      isOutput=True)

    dma_eng = [nc.sync, nc.scalar]  # both HWDGE queues

    with TileContext(nc) as tc:
        with tc.tile_pool(name="p", bufs=1) as pool, \
             tc.tile_pool(name="fp", bufs=1) as fpool, \
             tc.tile_pool(name="ps", bufs=1, space="PSUM") as ppool:

            # weight-pipeline inputs ride the gpsimd software-DGE queue so
            # their completion semaphores are independent of the big feat
            # loads on the two hardware queues.
            n_all = pool.tile([128, U, 3, XH], bf, tag="n_all")
            nc.scalar.dma_start(out=n_all[:], in_=d_nrm[:])
            nre = pool.tile([128, 3, XH], bf, tag="nre")
            nc.scalar.dma_start(out=nre[:], in_=d_nre[:])
            v_all = pool.tile([128, U, XH], bf, tag="v_all")
            nc.scalar.dma_start(out=v_all[:], in_=d_vld[:])

            # m*identity stationary tiles (host-sent): tap multiplicity folds
            # into the PE accumulation (both for Z and for the output MAC).
            idt = pool.tile([128, len(mults), 128], bf, tag="idt")
            nc.scalar.dma_start(out=idt[:],
                                in_=d_idm[:].rearrange("m p q -> p m q"))
            dma_rr = [0]
            id_m = {m: idt[:, mi, :] for mi, m in enumerate(mults)}

            # preload the ACT function tables off the critical path
            scr = pool.tile([128, 8], f32, tag="scr")
            nc.vector.memset(scr[:], 1.0)
            nc.scalar.activation(out=scr[:], in_=scr[:], func=Act.Sqrt)
            nc.scalar.activation(out=scr[:], in_=scr[:], func=Act.Exp)
            nc.scalar.activation(out=scr[:], in_=scr[:], func=Act.Copy)

            f_d = {}
            for i, v in enumerate(variants):
                f_d[v] = fpool.tile([128, C, XW], bf, tag=f"fd{i}",
                                    name=f"feat_v{i}")
                dma_eng[i % 2].dma_start(out=f_d[v][:], in_=d_feat[i])

            # ---- weight pipeline: one fat op per stage, split into two
            # x-halves so half 1's serial chain overlaps half 0's MAC ----
            HW2 = XH // 2
            d3 = pool.tile([128, U, 3, XH], bf, tag="d3")
            dsq = pool.tile([128, U, XH], bf, tag="dsq")
            t2 = pool.tile([128, U, XH], bf, tag="t2")
            nw = pool.tile([128, U, XH], bf, tag="nw")
            e_t = pool.tile([128, U, XH], bf, tag="e")
            r_t = pool.tile([128, XH], bf, tag="r")
            w_t = pool.tile([128, U, XH], bf, tag="w")
            zps = [ppool.tile([128, HW2], f32, tag=f"zps{h}", name=f"zps{h}")
                   for h in range(2)]

            def weights_half(h):
                xs = slice(h * HW2, (h + 1) * HW2)
                nc.vector.tensor_tensor(
                    out=d3[:, :, :, xs], in0=n_all[:, :, :, xs],
                    in1=nre[:, None, :, xs].broadcast_to([128, U, 3, HW2]),
                    op=Alu.subtract)
                nc.vector.tensor_tensor(out=dsq[:, :, xs], in0=d3[:, :, 0, xs],
                                        in1=d3[:, :, 0, xs], op=Alu.mult)
                nc.vector.tensor_tensor(out=t2[:, :, xs], in0=d3[:, :, 1, xs],
                                        in1=d3[:, :, 1, xs], op=Alu.mult)
                nc.vector.tensor_tensor(out=dsq[:, :, xs], in0=dsq[:, :, xs],
                                        in1=t2[:, :, xs], op=Alu.add)
                nc.vector.tensor_tensor(out=t2[:, :, xs], in0=d3[:, :, 2, xs],
                                        in1=d3[:, :, 2, xs], op=Alu.mult)
                nc.vector.tensor_tensor(out=dsq[:, :, xs], in0=dsq[:, :, xs],
                                        in1=t2[:, :, xs], op=Alu.add)
                nc.scalar.activation(out=dsq[:, :, xs], in_=dsq[:, :, xs],
                                     func=Act.Sqrt)
                nc.scalar.activation(out=nw[:, :, xs], in_=dsq[:, :, xs],
                                     func=Act.Exp, scale=-0.5)
                nc.vector.tensor_tensor(out=nw[:, :, xs], in0=v_all[:, :, xs],
                                        in1=nw[:, :, xs], op=Alu.mult)
                nc.scalar.activation(out=e_t[:, :, xs], in_=nw[:, :, xs],
                                     func=Act.Exp)
                # Z = sum_u m_u e_u on the PE; w_u = e_u / Z
                for k, (j, (dy, dx), m) in enumerate(ordered):
                    nc.tensor.matmul(zps[h][:], id_m[m], e_t[:, j, xs],
                                     start=(k == 0), stop=(k == U - 1))
                with nc.allow_low_precision(
                        reason="Z in [15, 41]; bf16 recip fine"):
                    nc.vector.reciprocal(out=r_t[:, xs], in_=zps[h][:])
                nc.vector.tensor_tensor(
                    out=w_t[:, :, xs], in0=e_t[:, :, xs],
                    in1=r_t[:, None, xs].broadcast_to([128, U, HW2]),
                    op=Alu.mult)

            weights_half(0)
            weights_half(1)

            # ---- MAC: DVE broadcast-multiplies; tap accumulation on the
            # TensorEngine as m*identity matmuls accumulating in PSUM ----
            QS = XH // 4                # 40-column PSUM quarters
            QF = C * QS                 # 1280 psum columns per quarter
            for half in range(2):       # PSUM capacity: 2 quarters per pass
                x0 = half * 2 * QS
                tmps = []
                for k, (j, (dy, dx), m) in enumerate(ordered):
                    par, xo = _tap_src(dx)
                    tmp = fpool.tile([128, 2, C, QS], bf, tag="tmp",
                                     name=f"tmp_{half}_{k}", bufs=4)
                    fsl = f_d[(dy, par)][:, :, xo + x0: xo + x0 + 2 * QS]
                    nc.vector.tensor_tensor(
                        out=tmp[:],
                        in0=fsl.rearrange("p c (q x) -> p q c x", q=2),
                        in1=w_t[:, j, x0:x0 + 2 * QS]
                            .rearrange("p (q x) -> p q x", q=2)[:, :, None, :]
                            .broadcast_to([128, 2, C, QS]),
                        op=Alu.mult)
                    tmps.append(tmp)
                pss = [ppool.tile([128, QF], f32, tag=f"ps{q}",
                                  name=f"ps_{half}_{q}") for q in range(2)]
                for k, (j, (dy, dx), m) in enumerate(ordered):
                    tf = tmps[k][:].rearrange("p q c x -> p (q c x)")
                    for q in range(2):
                        for s in range(0, QF, 512):
                            n = min(512, QF - s)
                            nc.tensor.matmul(
                                pss[q][:, s:s + n], id_m[m][:],
                                tf[:, q * QF + s: q * QF + s + n],
                                start=(k == 0), stop=(k == U - 1))
                for q in range(2):
                    oq = fpool.tile([128, QF], f32, tag=f"oq{q}",
                                    name=f"oq_{half}_{q}", bufs=2)
                    nc.scalar.activation(out=oq[:], in_=pss[q][:],
                                         func=Act.Copy)
                    dma_eng[dma_rr[0] % 2].dma_start(
                        out=d_out[half * 2 + q], in_=oq[:])
                    dma_rr[0] += 1

    nc.compile()
    return nc


def _prep_core_inputs(i, features, surface_normal, valid_f, taps, variants):
    """Host-side shard prep for core i -> dict of device arrays.

    Builds one contiguous [128, ...] image per (dy, parity) variant so each
    device load is a single dense DMA whose outer dim (128) sprays across
    all 16 DMA engines. Padded row yext <-> image row r0 - 4 + yext; padded
    col jj <-> image col jj - 4 (pad 4 left so every variant window is
    in-bounds).
    """
    b = i // 4
    r0 = (i % 4) * RCH
    lo = max(0, r0 - 4)
    hi = min(H, r0 + RCH + 4)
    ylo = lo - (r0 - 4)
    yhi = hi - (r0 - 4)

    fp = np.zeros((YEXT, C, PW), BF16)
    fp[ylo:yhi, :, 4:4 + W] = features[b, :, lo:hi, :].transpose(1, 0, 2)
    npd = np.zeros((YEXT, 3, PW), BF16)
    npd[ylo:yhi, :, 4:4 + W] = surface_normal[b, :, lo:hi, :].transpose(1, 0, 2)
    vp = np.zeros((YEXT, PW), BF16)
    vp[ylo:yhi, 4:4 + W] = valid_f[b, lo:hi, :]

    sn_view = surface_normal.reshape(B, H, W, 3)  # raw memory reinterpret
    clo = max(0, r0 - 2)
    chi = min(H, r0 + RCH + 2)
    nre_rows = np.zeros((ROWS, W, 3), np.float32)
    nre_rows[clo - (r0 - 2):chi - (r0 - 2)] = sn_view[b, clo:chi]
    nre = np.ascontiguousarray(np.concatenate(
        [nre_rows[:, xh * XH:(xh + 1) * XH, :].transpose(0, 2, 1)
         for xh in (0, 1)], 0)).astype(BF16)

    # feat variant (dy, par): tile[xh*64+y, ..., jj] = img[y + dy + 2, ...,
    # xh*XH + jj + par - 2] -> padded col offset xh*XH + par + 2.
    NV = len(variants)
    feat = np.empty((NV, 128, C, XW), BF16)
    for vi, (dy, par) in enumerate(variants):
        ys = dy + 2
        for xh in (0, 1):
            xs = xh * XH + par + 2
            feat[vi, xh * ROWS:(xh + 1) * ROWS] = \
                fp[ys:ys + ROWS, :, xs:xs + XW]
    # weight-pipeline inputs: fully-shifted per-tap images
    U = len(taps)
    nrm = np.empty((128, U, 3, XH), BF16)
    vld = np.empty((128, U, XH), BF16)
    for u, ((dy, dx), m) in enumerate(taps):
        ys = dy + 2
        for xh in (0, 1):
            xs = xh * XH + dx + 4
            nrm[xh * ROWS:(xh + 1) * ROWS, u] = \
                npd[ys:ys + ROWS, :, xs:xs + XH]
            vld[xh * ROWS:(xh + 1) * ROWS, u] = vp[ys:ys + ROWS, xs:xs + XH]
    return {"feat": feat, "nrm": nrm, "vld": vld, "nre": nre}


def _run_device(inputs, trace=False):
    features = np.ascontiguousarray(np.asarray(inputs["features"], np.float32))
    surface_normal = np.ascontiguousarray(
        np.asarray(inputs["surface_normal"], np.float32))
    depth = np.asarray(inputs["depth"], np.float32)
    sample_idx = np.asarray(inputs["sample_idx"])

    d = depth[:, 0]
    valid_f = ((d > 0) & (d < DEPTH_MAX)).astype(np.float32)

    taps = _unique_taps(sample_idx)
    if taps not in _compiled:
        _compiled[taps] = _build(taps)
    nc = _compiled[taps]

    variants = _variants(taps)
    mults = sorted({m for _, m in taps})
    idm = np.stack([np.eye(128, dtype=np.float32) * m
                    for m in mults]).astype(BF16)
    in_maps = []
    for i in range(NCORES):
        m = _prep_core_inputs(i, features, surface_normal, valid_f, taps,
                              variants)
        m["idm"] = idm
        in_maps.append(m)
    res = run_bass_kernel_spmd(nc, in_maps, list(range(NCORES)), trace=trace)

    out = np.empty((B, C, H, W), np.float32)
    for i in range(NCORES):
        b = i // 4
        r0 = (i % 4) * RCH
        o = np.asarray(res.results[i]["out"], np.float32)  # [4,128,C,QS]
        QS = XH // 4
        for q in range(4):
            for xh in (0, 1):
                sl = o[q, xh * ROWS + 2: xh * ROWS + 2 + RCH]  # [RCH, C, QS]
                out[b, :, r0:r0 + RCH,
                    xh * XH + q * QS: xh * XH + (q + 1) * QS] = \
                    sl.transpose(1, 0, 2)
    return out, res


def _reference_numpy(depth, surface_normal, features, guide_weight, sample_idx):
    """Plain-numpy port of the reference (general fallback path)."""
    b, c, h, w = features.shape
    d = depth[:, 0]
    valid = ((d > 0) & (d < DEPTH_MAX)).astype(features.dtype)[:, None]

    def gather(x):
        B_, C_, H_, W_ = x.shape
        xp = np.pad(x, ((0, 0), (0, 0), (2, 2), (2, 2)))
        slabs = []
        for i in range(SAMPLE_NUM):
            p = int(sample_idx[i])
            dy, dx = p // K_SIZE, p % K_SIZE
            slabs.append(xp[:, :, dy:dy + H_, dx:dx + W_])
        return np.stack(slabs, 1).transpose(0, 3, 4, 1, 2)  # [B,H,W,S,C]

    feat_s = gather(features)
    norm_s = gather(surface_normal)
    valid_s = gather(valid)[..., 0]
    center_n = surface_normal.reshape(b, h, w, 3)
    diff = np.sqrt(((norm_s - center_n[:, :, :, None, :]) ** 2).sum(-1))
    normal_w = np.exp(-0.5 * diff)
    guide_s = guide_weight[..., np.asarray(sample_idx)]
    fw = valid_s * normal_w * guide_s
    fw = fw - fw.max(-1, keepdims=True)
    fw = np.exp(fw)
    fw = fw / fw.sum(-1, keepdims=True)
    out = (feat_s * fw[..., None]).sum(3)
    return out.transpose(0, 3, 1, 2).astype(features.dtype)


def kernel(**inputs):
    features = np.asarray(inputs["features"])
    guide = np.asarray(inputs["guide_weight"])
    if not np.all(guide == 1.0):
        # General path (never taken for this problem's spec: fill=ones).
        out = _reference_numpy(
            np.asarray(inputs["depth"], np.float32),
            np.ascontiguousarray(np.asarray(inputs["surface_normal"], np.float32)),
            np.ascontiguousarray(np.asarray(inputs["features"], np.float32)),
            np.asarray(guide, np.float32),
            np.asarray(inputs["sample_idx"]))
        return out, features
    out, _ = _run_device(inputs)
    return out, features


if __name__ == "__main__":
    rng = np.random.default_rng(0)
    inputs = {
        "depth": rng.uniform(0, 200, (B, 1, H, W)).astype(np.float32),
        "surface_normal": rng.standard_normal((B, 3, H, W)).astype(np.float32),
        "features": rng.standard_normal((B, C, H, W)).astype(np.float32),
        "guide_weight": np.ones((B, H, W, 25), np.float32),
        "sample_idx": rng.integers(0, 25, 15).astype(np.int32),
    }
    out, _ = kernel(**inputs)
    exp = _reference_numpy(
        inputs["depth"], inputs["surface_normal"], inputs["features"],
        inputs["guide_weight"], inputs["sample_idx"])
    err = np.linalg.norm(out - exp) / np.linalg.norm(exp)
    print("smoke rel err:", err)


# revision 29
# speedup vs baseline: 1.0100x; 1.0100x over previous
"""Trainium2 Bass kernel for nn_AdaptiveSample (per-pixel 5x5 sampled softmax
aggregation), distributed over 8 NeuronCores.

Sharding: data-parallel over (batch, H): core i handles batch i//4, rows
[60*(i%4), 60*(i%4)+60). Halo rows are read directly from the full input on
the host (full_io), so no device collectives are needed.

Device layout: partitions = (x-half, row) -> 2*64 = 128 partitions per core
(60 owned rows + 2+2 halo rows per x-half). Free dim = (channel, x) with a
column halo. dx taps become free-dim offsets; dy taps are handled by loading
dy-shifted copies of the inputs straight from DRAM (compute engines cannot
start at arbitrary partitions, DMA can read any DRAM rows). The weighted sum
runs on the VectorEngine in bf16 (2x mode); transcendentals on ScalarEngine.
Even/odd-dx copies keep bf16 operands 4-byte aligned for the 2x DVE mode.

sample_idx is read on the host at call time and the kernel is compiled for
the unique (dy, dx) taps with multiplicities folded into the exp bias
(exp(x + ln m) = m*exp(x)).

guide_weight is all-ones per the problem spec; this is verified at runtime
and a numpy fallback handles the general case.
"""

import os
import sys

for _p in ("/opt/trn_rl_repo", "/root/.axon_site/_ro/trn_rl_repo"):
    if os.path.isdir(_p) and _p not in sys.path:
        sys.path.insert(0, _p)

import numpy as np
import ml_dtypes

import concourse.bass as bass
import concourse.bacc as bacc
import concourse.mybir as mybir
from concourse.tile import TileContext
from concourse.bass_utils import run_bass_kernel_spmd
from concourse.masks import make_identity

BF16 = ml_dtypes.bfloat16

K_SIZE = 5
SAMPLE_NUM = 15
DEPTH_MAX = 192.0

B, C, H, W = 2, 32, 240, 320
NCORES = 8
RCH = H * B // NCORES          # 60 owned rows per core
ROWS = RCH + 4                 # 64 rows incl. dy halo
YEXT = ROWS + 4                # 68 DRAM rows (dy-shifted loads need +-2 more)
XH = W // 2                    # 160: x is split in half across partitions
XW = XH + 4                    # 164: x window incl. dx halo
XD = XW + 4                    # 168 DRAM cols (parity-shifted loads)
PW = W + 10                    # padded row width for host prep

_compiled = {}


def _unique_taps(sample_idx):
    """-> sorted tuple of ((dy, dx), mult), dy/dx in [-2, 2]."""
    from collections import Counter
    cnt = Counter()
    for p in np.asarray(sample_idx).tolist():
        cnt[(p // K_SIZE - 2, p % K_SIZE - 2)] += 1
    return tuple(sorted(cnt.items()))


def _tap_src(dx):
    """-> (parity, x-offset) for a 160-wide slice of a parity tile."""
    par = dx & 1
    return par, 2 + dx - par


def _variants(taps):
    """Distinct (dy, parity) variant list, in tap (dy-sorted) order."""
    seen = []
    for (dy, dx), _ in taps:
        v = (dy, dx & 1)
        if v not in seen:
            seen.append(v)
    return seen


def _build(taps):
    """Build the per-core Bass program for the given unique taps."""
    U = len(taps)
    f32 = mybir.dt.float32
    bf = mybir.dt.bfloat16
    Alu = mybir.AluOpType
    Act = mybir.ActivationFunctionType

    dys = sorted({dy for (dy, _), _ in taps})
    by_dy = {d: [(j, (dy, dx), m) for j, ((dy, dx), m) in enumerate(taps)
                 if dy == d] for d in dys}
    variants = _variants(taps)
    vidx = {v: i for i, v in enumerate(variants)}
    NV = len(variants)

    nc = bacc.Bacc()

    ordered = [(j, (dy, dx), m) for dy in dys
               for j, (dy, dx), m in by_dy[dy]]
    mults = sorted({m for _, _, m in ordered})

    # Per-tap fully-shifted weight inputs (one fat op per pipeline stage);
    # per-(dy,parity) variant feature images for the MAC (dx via slices).
    d_feat = nc.declare_dram_parameter("feat", [NV, 128, C, XW], bf,
                                       isOutput=False)
    d_nrm = nc.declare_dram_parameter("nrm", [128, U, 3, XH], bf,
                                      isOutput=False)
    d_vld = nc.declare_dram_parameter("vld", [128, U, XH], bf,
                                      isOutput=False)
    d_nre = nc.declare_dram_parameter("nre", [128, 3, XH], bf, isOutput=False)
    d_idm = nc.declare_dram_parameter("idm", [len(mults), 128, 128], bf,
                                      isOutput=False)
    d_out = nc.declare_dram_parameter("out", [4, 128, C, XH // 4], f32,
                                      isOutput=True)

    dma_eng = [nc.sync, nc.scalar]  # both HWDGE queues

    with TileContext(nc) as tc:
        with tc.tile_pool(name="p", bufs=1) as pool, \
             tc.tile_pool(name="fp", bufs=1) as fpool, \
             tc.tile_pool(name="ps", bufs=1, space="PSUM") as ppool:

            # weight-pipeline inputs ride the gpsimd software-DGE queue so
            # their completion semaphores are independent of the big feat
            # loads on the two hardware queues.
            n_all = pool.tile([128, U, 3, XH], bf, tag="n_all")
            nc.scalar.dma_start(out=n_all[:], in_=d_nrm[:])
            nre = pool.tile([128, 3, XH], bf, tag="nre")
            nc.scalar.dma_start(out=nre[:], in_=d_nre[:])
            v_all = pool.tile([128, U, XH], bf, tag="v_all")
            nc.scalar.dma_start(out=v_all[:], in_=d_vld[:])

            # m*identity stationary tiles (host-sent): tap multiplicity folds
            # into the PE accumulation (both for Z and for the output MAC).
            idt = pool.tile([128, len(mults), 128], bf, tag="idt")
            nc.scalar.dma_start(out=idt[:],
                                in_=d_idm[:].rearrange("m p q -> p m q"))
            dma_rr = [0]
            id_m = {m: idt[:, mi, :] for mi, m in enumerate(mults)}

            # preload the ACT function tables off the critical path
            scr = pool.tile([128, 8], f32, tag="scr")
            nc.vector.memset(scr[:], 1.0)
            nc.scalar.activation(out=scr[:], in_=scr[:], func=Act.Sqrt)
            nc.scalar.activation(out=scr[:], in_=scr[:], func=Act.Exp)
            nc.scalar.activation(out=scr[:], in_=scr[:], func=Act.Copy)

            f_d = {}
            for i, v in enumerate(variants):
                f_d[v] = fpool.tile([128, C, XW], bf, tag=f"fd{i}",
                                    name=f"feat_v{i}")
                dma_eng[i % 2].dma_start(out=f_d[v][:], in_=d_feat[i])

            # ---- weight pipeline: one fat op per stage, split into two
            # x-halves so half 1's serial chain overlaps half 0's MAC ----
            HW2 = XH // 2
            d3 = pool.tile([128, U, 3, XH], bf, tag="d3")
            dsq = pool.tile([128, U, XH], bf, tag="dsq")
            t2 = pool.tile([128, U, XH], bf, tag="t2")
            nw = pool.tile([128, U, XH], bf, tag="nw")
            e_t = pool.tile([128, U, XH], bf, tag="e")
            r_t = pool.tile([128, XH], bf, tag="r")
            w_t = pool.tile([128, U, XH], bf, tag="w")
            zps = [ppool.tile([128, HW2], f32, tag=f"zps{h}", name=f"zps{h}")
                   for h in range(2)]

            def weights_half(h):
                xs = slice(h * HW2, (h + 1) * HW2)
                nc.vector.tensor_tensor(
                    out=d3[:, :, :, xs], in0=n_all[:, :, :, xs],
                    in1=nre[:, None, :, xs].broadcast_to([128, U, 3, HW2]),
                    op=Alu.subtract)
                nc.vector.tensor_tensor(out=dsq[:, :, xs], in0=d3[:, :, 0, xs],
                                        in1=d3[:, :, 0, xs], op=Alu.mult)
                nc.vector.tensor_tensor(out=t2[:, :, xs], in0=d3[:, :, 1, xs],
                                        in1=d3[:, :, 1, xs], op=Alu.mult)
                nc.vector.tensor_tensor(out=dsq[:, :, xs], in0=dsq[:, :, xs],
                                        in1=t2[:, :, xs], op=Alu.add)
                nc.vector.tensor_tensor(out=t2[:, :, xs], in0=d3[:, :, 2, xs],
                                        in1=d3[:, :, 2, xs], op=Alu.mult)
                nc.vector.tensor_tensor(out=dsq[:, :, xs], in0=dsq[:, :, xs],
                                        in1=t2[:, :, xs], op=Alu.add)
                nc.scalar.activation(out=dsq[:, :, xs], in_=dsq[:, :, xs],
                                     func=Act.Sqrt)
                nc.scalar.activation(out=nw[:, :, xs], in_=dsq[:, :, xs],
                                     func=Act.Exp, scale=-0.5)
                nc.vector.tensor_tensor(out=nw[:, :, xs], in0=v_all[:, :, xs],
                                        in1=nw[:, :, xs], op=Alu.mult)
                nc.scalar.activation(out=e_t[:, :, xs], in_=nw[:, :, xs],
                                     func=Act.Exp)
                # Z = sum_u m_u e_u on the PE; w_u = e_u / Z
                for k, (j, (dy, dx), m) in enumerate(ordered):
                    nc.tensor.matmul(zps[h][:], id_m[m], e_t[:, j, xs],
                                     start=(k == 0), stop=(k == U - 1))
                with nc.allow_low_precision(
                        reason="Z in [15, 41]; bf16 recip fine"):
                    nc.vector.reciprocal(out=r_t[:, xs], in_=zps[h][:])
                nc.vector.tensor_tensor(
                    out=w_t[:, :, xs], in0=e_t[:, :, xs],
                    in1=r_t[:, None, xs].broadcast_to([128, U, HW2]),
                    op=Alu.mult)

            weights_half(0)
            weights_half(1)

            # ---- MAC: DVE broadcast-multiplies; tap accumulation on the
            # TensorEngine as m*identity matmuls accumulating in PSUM ----
            QS = XH // 4                # 40-column PSUM quarters
            QF = C * QS                 # 1280 psum columns per quarter
            for half in range(2):       # PSUM capacity: 2 quarters per pass
                x0 = half * 2 * QS
                tmps = []
                for k, (j, (dy, dx), m) in enumerate(ordered):
                    par, xo = _tap_src(dx)
                    tmp = fpool.tile([128, 2, C, QS], bf, tag="tmp",
                                     name=f"tmp_{half}_{k}", bufs=4)
                    fsl = f_d[(dy, par)][:, :, xo + x0: xo + x0 + 2 * QS]
                    nc.vector.tensor_tensor(
                        out=tmp[:],
                        in0=fsl.rearrange("p c (q x) -> p q c x", q=2),
                        in1=w_t[:, j, x0:x0 + 2 * QS]
                            .rearrange("p (q x) -> p q x", q=2)[:, :, None, :]
                            .broadcast_to([128, 2, C, QS]),
                        op=Alu.mult)
                    tmps.append(tmp)
                pss = [ppool.tile([128, QF], f32, tag=f"ps{q}",
                                  name=f"ps_{half}_{q}") for q in range(2)]
                for k, (j, (dy, dx), m) in enumerate(ordered):
                    tf = tmps[k][:].rearrange("p q c x -> p (q c x)")
                    for q in range(2):
                        for s in range(0, QF, 512):
                            n = min(512, QF - s)
                            nc.tensor.matmul(
                                pss[q][:, s:s + n], id_m[m][:],
                                tf[:, q * QF + s: q * QF + s + n],
                                start=(k == 0), stop=(k == U - 1))
                for q in range(2):
                    oq = fpool.tile([128, QF], f32, tag=f"oq{q}",
                                    name=f"oq_{half}_{q}", bufs=2)
                    nc.scalar.activation(out=oq[:], in_=pss[q][:],
                                         func=Act.Copy)
                    nc.sync.dma_start(out=d_out[half * 2 + q], in_=oq[:])

    nc.compile()
    return nc


def _prep_core_inputs(i, features, surface_normal, valid_f, taps, variants):
    """Host-side shard prep for core i -> dict of device arrays.

    Builds one contiguous [128, ...] image per (dy, parity) variant so each
    device load is a single dense DMA whose outer dim (128) sprays across
    all 16 DMA engines. Padded row yext <-> image row r0 - 4 + yext; padded
    col jj <-> image col jj - 4 (pad 4 left so every variant window is
    in-bounds).
    """
    b = i // 4
    r0 = (i % 4) * RCH
    lo = max(0, r0 - 4)
    hi = min(H, r0 + RCH + 4)
    ylo = lo - (r0 - 4)
    yhi = hi - (r0 - 4)

    fp = np.zeros((YEXT, C, PW), BF16)
    fp[ylo:yhi, :, 4:4 + W] = features[b, :, lo:hi, :].transpose(1, 0, 2)
    npd = np.zeros((YEXT, 3, PW), BF16)
    npd[ylo:yhi, :, 4:4 + W] = surface_normal[b, :, lo:hi, :].transpose(1, 0, 2)
    vp = np.zeros((YEXT, PW), BF16)
    vp[ylo:yhi, 4:4 + W] = valid_f[b, lo:hi, :]

    sn_view = surface_normal.reshape(B, H, W, 3)  # raw memory reinterpret
    clo = max(0, r0 - 2)
    chi = min(H, r0 + RCH + 2)
    nre_rows = np.zeros((ROWS, W, 3), np.float32)
    nre_rows[clo - (r0 - 2):chi - (r0 - 2)] = sn_view[b, clo:chi]
    nre = np.ascontiguousarray(np.concatenate(
        [nre_rows[:, xh * XH:(xh + 1) * XH, :].transpose(0, 2, 1)
         for xh in (0, 1)], 0)).astype(BF16)

    # feat variant (dy, par): tile[xh*64+y, ..., jj] = img[y + dy + 2, ...,
    # xh*XH + jj + par - 2] -> padded col offset xh*XH + par + 2.
    NV = len(variants)
    feat = np.empty((NV, 128, C, XW), BF16)
    for vi, (dy, par) in enumerate(variants):
        ys = dy + 2
        for xh in (0, 1):
            xs = xh * XH + par + 2
            feat[vi, xh * ROWS:(xh + 1) * ROWS] = \
                fp[ys:ys + ROWS, :, xs:xs + XW]
    # weight-pipeline inputs: fully-shifted per-tap images
    U = len(taps)
    nrm = np.empty((128, U, 3, XH), BF16)
    vld = np.empty((128, U, XH), BF16)
    for u, ((dy, dx), m) in enumerate(taps):
        ys = dy + 2
        for xh in (0, 1):
            xs = xh * XH + dx + 4
            nrm[xh * ROWS:(xh + 1) * ROWS, u] = \
                npd[ys:ys + ROWS, :, xs:xs + XH]
            vld[xh * ROWS:(xh + 1) * ROWS, u] = vp[ys:ys + ROWS, xs:xs + XH]
    return {"feat": feat, "nrm": nrm, "vld": vld, "nre": nre}


def _run_device(inputs, trace=False):
    features = np.ascontiguousarray(np.asarray(inputs["features"], np.float32))
    surface_normal = np.ascontiguousarray(
        np.asarray(inputs["surface_normal"], np.float32))
    depth = np.asarray(inputs["depth"], np.float32)
    sample_idx = np.asarray(inputs["sample_idx"])

    d = depth[:, 0]
    valid_f = ((d > 0) & (d < DEPTH_MAX)).astype(np.float32)

    taps = _unique_taps(sample_idx)
    if taps not in _compiled:
        _compiled[taps] = _build(taps)
    nc = _compiled[taps]

    variants = _variants(taps)
    mults = sorted({m for _, m in taps})
    idm = np.stack([np.eye(128, dtype=np.float32) * m
                    for m in mults]).astype(BF16)
    in_maps = []
    for i in range(NCORES):
        m = _prep_core_inputs(i, features, surface_normal, valid_f, taps,
                              variants)
        m["idm"] = idm
        in_maps.append(m)
    res = run_bass_kernel_spmd(nc, in_maps, list(range(NCORES)), trace=trace)

    out = np.empty((B, C, H, W), np.float32)
    for i in range(NCORES):
        b = i // 4
        r0 = (i % 4) * RCH
        o = np.asarray(res.results[i]["out"], np.float32)  # [4,128,C,QS]
        QS = XH // 4
        for q in range(4):
            for xh in (0, 1):
                sl = o[q, xh * ROWS + 2: xh * ROWS + 2 + RCH]  # [RCH, C, QS]
                out[b, :, r0:r0 + RCH,
                    xh * XH + q * QS: xh * XH + (q + 1) * QS] = \
                    sl.transpose(1, 0, 2)
    return out, res


def _reference_numpy(depth, surface_normal, features, guide_weight, sample_idx):
    """Plain-numpy port of the reference (general fallback path)."""
    b, c, h, w = features.shape
    d = depth[:, 0]
    valid = ((d > 0) & (d < DEPTH_MAX)).astype(features.dtype)[:, None]

    def gather(x):
        B_, C_, H_, W_ = x.shape
        xp = np.pad(x, ((0, 0), (0, 0), (2, 2), (2, 2)))
        slabs = []
        for i in range(SAMPLE_NUM):
            p = int(sample_idx[i])
            dy, dx = p // K_SIZE, p % K_SIZE
            slabs.append(xp[:, :, dy:dy + H_, dx:dx + W_])
        return np.stack(slabs, 1).transpose(0, 3, 4, 1, 2)  # [B,H,W,S,C]

    feat_s = gather(features)
    norm_s = gather(surface_normal)
    valid_s = gather(valid)[..., 0]
    center_n = surface_normal.reshape(b, h, w, 3)
    diff = np.sqrt(((norm_s - center_n[:, :, :, None, :]) ** 2).sum(-1))
    normal_w = np.exp(-0.5 * diff)
    guide_s = guide_weight[..., np.asarray(sample_idx)]
    fw = valid_s * normal_w * guide_s
    fw = fw - fw.max(-1, keepdims=True)
    fw = np.exp(fw)
    fw = fw / fw.sum(-1, keepdims=True)
    out = (feat_s * fw[..., None]).sum(3)
    return out.transpose(0, 3, 1, 2).astype(features.dtype)


def kernel(**inputs):
    features = np.asarray(inputs["features"])
    guide = np.asarray(inputs["guide_weight"])
    if not np.all(guide == 1.0):
        # General path (never taken for this problem's spec: fill=ones).
        out = _reference_numpy(
            np.asarray(inputs["depth"], np.float32),
            np.ascontiguousarray(np.asarray(inputs["surface_normal"], np.float32)),
            np.ascontiguousarray(np.asarray(inputs["features"], np.float32)),
            np.asarray(guide, np.float32),
            np.asarray(inputs["sample_idx"]))
        return out, features
    out, _ = _run_device(inputs)
    return out, features


if __name__ == "__main__":
    rng = np.random.default_rng(0)
    inputs = {
        "depth": rng.uniform(0, 200, (B, 1, H, W)).astype(np.float32),
        "surface_normal": rng.standard_normal((B, 3, H, W)).astype(np.float32),
        "features": rng.standard_normal((B, C, H, W)).astype(np.float32),
        "guide_weight": np.ones((B, H, W, 25), np.float32),
        "sample_idx": rng.integers(0, 25, 15).astype(np.int32),
    }
    out, _ = kernel(**inputs)
    exp = _reference_numpy(
        inputs["depth"], inputs["surface_normal"], inputs["features"],
        inputs["guide_weight"], inputs["sample_idx"])
    err = np.linalg.norm(out - exp) / np.linalg.norm(exp)
    print("smoke rel err:", err)


# revision 30
# speedup vs baseline: 1.1570x; 1.1455x over previous
"""Trainium2 Bass kernel for nn_AdaptiveSample (per-pixel 5x5 sampled softmax
aggregation), distributed over 8 NeuronCores.

Sharding: data-parallel over (batch, H): core i handles batch i//4, rows
[60*(i%4), 60*(i%4)+60). Halo rows are read directly from the full input on
the host (full_io), so no device collectives are needed.

Device layout: partitions = (x-half, row) -> 2*64 = 128 partitions per core
(60 owned rows + 2+2 halo rows per x-half). Free dim = (channel, x) with a
column halo. dx taps become free-dim offsets; dy taps are handled by loading
dy-shifted copies of the inputs straight from DRAM (compute engines cannot
start at arbitrary partitions, DMA can read any DRAM rows). The weighted sum
runs on the VectorEngine in bf16 (2x mode); transcendentals on ScalarEngine.
Even/odd-dx copies keep bf16 operands 4-byte aligned for the 2x DVE mode.

sample_idx is read on the host at call time and the kernel is compiled for
the unique (dy, dx) taps with multiplicities folded into the exp bias
(exp(x + ln m) = m*exp(x)).

guide_weight is all-ones per the problem spec; this is verified at runtime
and a numpy fallback handles the general case.
"""

import os
import sys

for _p in ("/opt/trn_rl_repo", "/root/.axon_site/_ro/trn_rl_repo"):
    if os.path.isdir(_p) and _p not in sys.path:
        sys.path.insert(0, _p)

import numpy as np
import ml_dtypes

import concourse.bass as bass
import concourse.bacc as bacc
import concourse.mybir as mybir
from concourse.tile import TileContext
from concourse.bass_utils import run_bass_kernel_spmd
from concourse.masks import make_identity

BF16 = ml_dtypes.bfloat16

K_SIZE = 5
SAMPLE_NUM = 15
DEPTH_MAX = 192.0

B, C, H, W = 2, 32, 240, 320
NCORES = 8
RCH = H * B // NCORES          # 60 owned rows per core
ROWS = RCH + 4                 # 64 rows incl. dy halo
YEXT = ROWS + 4                # 68 DRAM rows (dy-shifted loads need +-2 more)
XH = W // 2                    # 160: x is split in half across partitions
XW = XH + 4                    # 164: x window incl. dx halo
XD = XW + 4                    # 168 DRAM cols (parity-shifted loads)
PW = W + 10                    # padded row width for host prep

_compiled = {}


def _unique_taps(sample_idx):
    """-> sorted tuple of ((dy, dx), mult), dy/dx in [-2, 2]."""
    from collections import Counter
    cnt = Counter()
    for p in np.asarray(sample_idx).tolist():
        cnt[(p // K_SIZE - 2, p % K_SIZE - 2)] += 1
    return tuple(sorted(cnt.items()))


def _tap_src(dx):
    """-> (parity, x-offset) for a 160-wide slice of a parity tile."""
    par = dx & 1
    return par, 2 + dx - par


def _variants(taps):
    """Distinct (dy, parity) variant list, in tap (dy-sorted) order."""
    seen = []
    for (dy, dx), _ in taps:
        v = (dy, dx & 1)
        if v not in seen:
            seen.append(v)
    return seen


def _build(taps):
    """Build the per-core Bass program for the given unique taps."""
    U = len(taps)
    f32 = mybir.dt.float32
    bf = mybir.dt.bfloat16
    Alu = mybir.AluOpType
    Act = mybir.ActivationFunctionType

    dys = sorted({dy for (dy, _), _ in taps})
    by_dy = {d: [(j, (dy, dx), m) for j, ((dy, dx), m) in enumerate(taps)
                 if dy == d] for d in dys}
    variants = _variants(taps)
    vidx = {v: i for i, v in enumerate(variants)}
    NV = len(variants)

    nc = bacc.Bacc()

    ordered = [(j, (dy, dx), m) for dy in dys
               for j, (dy, dx), m in by_dy[dy]]
    mults = sorted({m for _, _, m in ordered})

    # Per-tap fully-shifted weight inputs (one fat op per pipeline stage);
    # per-(dy,parity) variant feature images for the MAC (dx via slices).
    d_feat = nc.declare_dram_parameter("feat", [NV, 128, C, XW], bf,
                                       isOutput=False)
    d_nrm = nc.declare_dram_parameter("nrm", [128, U, 3, XH], bf,
                                      isOutput=False)
    d_vld = nc.declare_dram_parameter("vld", [128, U, XH], bf,
                                      isOutput=False)
    d_nre = nc.declare_dram_parameter("nre", [128, 3, XH], bf, isOutput=False)
    d_idm = nc.declare_dram_parameter("idm", [len(mults), 128, 128], bf,
                                      isOutput=False)
    d_out = nc.declare_dram_parameter("out", [4, 128, C, XH // 4], f32,
                                      isOutput=True)

    dma_eng = [nc.sync, nc.scalar]  # both HWDGE queues

    with TileContext(nc) as tc:
        with tc.tile_pool(name="p", bufs=1) as pool, \
             tc.tile_pool(name="fp", bufs=1) as fpool, \
             tc.tile_pool(name="ps", bufs=1, space="PSUM") as ppool:

            # weight-pipeline inputs ride the gpsimd software-DGE queue so
            # their completion semaphores are independent of the big feat
            # loads on the two hardware queues.
            nre = pool.tile([128, 3, XH], bf, tag="nre")
            nc.scalar.dma_start(out=nre[:], in_=d_nre[:])
            n_all = pool.tile([128, U, 3, XH], bf, tag="n_all")
            nc.scalar.dma_start(out=n_all[:], in_=d_nrm[:])
            v_all = pool.tile([128, U, XH], bf, tag="v_all")
            nc.scalar.dma_start(out=v_all[:], in_=d_vld[:])

            # m*identity stationary tiles (host-sent): tap multiplicity folds
            # into the PE accumulation (both for Z and for the output MAC).
            idt = pool.tile([128, len(mults), 128], bf, tag="idt")
            nc.scalar.dma_start(out=idt[:],
                                in_=d_idm[:].rearrange("m p q -> p m q"))
            id_m = {m: idt[:, mi, :] for mi, m in enumerate(mults)}

            # preload the ACT function tables off the critical path
            scr = pool.tile([128, 8], f32, tag="scr")
            nc.vector.memset(scr[:], 1.0)
            nc.scalar.activation(out=scr[:], in_=scr[:], func=Act.Sqrt)
            nc.scalar.activation(out=scr[:], in_=scr[:], func=Act.Exp)
            nc.scalar.activation(out=scr[:], in_=scr[:], func=Act.Copy)

            f_d = {}
            for i, v in enumerate(variants):
                f_d[v] = fpool.tile([128, C, XW], bf, tag=f"fd{i}",
                                    name=f"feat_v{i}")
                dma_eng[i % 2].dma_start(out=f_d[v][:], in_=d_feat[i])

            # ---- weight pipeline: one fat op per stage, split into two
            # x-halves so half 1's serial chain overlaps half 0's MAC ----
            HW2 = XH // 2
            d3 = pool.tile([128, U, 3, XH], bf, tag="d3")
            dsq = pool.tile([128, U, XH], bf, tag="dsq")
            t2 = pool.tile([128, U, XH], bf, tag="t2")
            nw = pool.tile([128, U, XH], bf, tag="nw")
            e_t = pool.tile([128, U, XH], bf, tag="e")
            r_t = pool.tile([128, XH], bf, tag="r")
            w_t = pool.tile([128, U, XH], bf, tag="w")
            zps = [ppool.tile([128, HW2], f32, tag=f"zps{h}", name=f"zps{h}")
                   for h in range(2)]

            def weights_half(h):
                xs = slice(h * HW2, (h + 1) * HW2)
                nc.vector.tensor_tensor(
                    out=d3[:, :, :, xs], in0=n_all[:, :, :, xs],
                    in1=nre[:, None, :, xs].broadcast_to([128, U, 3, HW2]),
                    op=Alu.subtract)
                nc.vector.tensor_tensor(out=dsq[:, :, xs], in0=d3[:, :, 0, xs],
                                        in1=d3[:, :, 0, xs], op=Alu.mult)
                nc.vector.tensor_tensor(out=t2[:, :, xs], in0=d3[:, :, 1, xs],
                                        in1=d3[:, :, 1, xs], op=Alu.mult)
                nc.vector.tensor_tensor(out=dsq[:, :, xs], in0=dsq[:, :, xs],
                                        in1=t2[:, :, xs], op=Alu.add)
                nc.vector.tensor_tensor(out=t2[:, :, xs], in0=d3[:, :, 2, xs],
                                        in1=d3[:, :, 2, xs], op=Alu.mult)
                nc.vector.tensor_tensor(out=dsq[:, :, xs], in0=dsq[:, :, xs],
                                        in1=t2[:, :, xs], op=Alu.add)
                nc.scalar.activation(out=dsq[:, :, xs], in_=dsq[:, :, xs],
                                     func=Act.Sqrt)
                nc.scalar.activation(out=nw[:, :, xs], in_=dsq[:, :, xs],
                                     func=Act.Exp, scale=-0.5)
                nc.vector.tensor_tensor(out=nw[:, :, xs], in0=v_all[:, :, xs],
                                        in1=nw[:, :, xs], op=Alu.mult)
                nc.scalar.activation(out=e_t[:, :, xs], in_=nw[:, :, xs],
                                     func=Act.Exp)
                # Z = sum_u m_u e_u on the PE; w_u = e_u / Z
                for k, (j, (dy, dx), m) in enumerate(ordered):
                    nc.tensor.matmul(zps[h][:], id_m[m], e_t[:, j, xs],
                                     start=(k == 0), stop=(k == U - 1))
                with nc.allow_low_precision(
                        reason="Z in [15, 41]; bf16 recip fine"):
                    nc.vector.reciprocal(out=r_t[:, xs], in_=zps[h][:])
                nc.vector.tensor_tensor(
                    out=w_t[:, :, xs], in0=e_t[:, :, xs],
                    in1=r_t[:, None, xs].broadcast_to([128, U, HW2]),
                    op=Alu.mult)

            weights_half(0)
            weights_half(1)

            # ---- MAC: DVE broadcast-multiplies; tap accumulation on the
            # TensorEngine as m*identity matmuls accumulating in PSUM ----
            QS = XH // 4                # 40-column PSUM quarters
            QF = C * QS                 # 1280 psum columns per quarter
            for half in range(2):       # PSUM capacity: 2 quarters per pass
                x0 = half * 2 * QS
                tmps = []
                for k, (j, (dy, dx), m) in enumerate(ordered):
                    par, xo = _tap_src(dx)
                    tmp = fpool.tile([128, 2, C, QS], bf, tag="tmp",
                                     name=f"tmp_{half}_{k}", bufs=4)
                    fsl = f_d[(dy, par)][:, :, xo + x0: xo + x0 + 2 * QS]
                    nc.vector.tensor_tensor(
                        out=tmp[:],
                        in0=fsl.rearrange("p c (q x) -> p q c x", q=2),
                        in1=w_t[:, j, x0:x0 + 2 * QS]
                            .rearrange("p (q x) -> p q x", q=2)[:, :, None, :]
                            .broadcast_to([128, 2, C, QS]),
                        op=Alu.mult)
                    tmps.append(tmp)
                pss = [ppool.tile([128, QF], f32, tag=f"ps{q}",
                                  name=f"ps_{half}_{q}") for q in range(2)]
                for k, (j, (dy, dx), m) in enumerate(ordered):
                    tf = tmps[k][:].rearrange("p q c x -> p (q c x)")
                    for q in range(2):
                        for s in range(0, QF, 512):
                            n = min(512, QF - s)
                            nc.tensor.matmul(
                                pss[q][:, s:s + n], id_m[m][:],
                                tf[:, q * QF + s: q * QF + s + n],
                                start=(k == 0), stop=(k == U - 1))
                for q in range(2):
                    oq = fpool.tile([128, QF], f32, tag=f"oq{q}",
                                    name=f"oq_{half}_{q}", bufs=2)
                    nc.scalar.activation(out=oq[:], in_=pss[q][:],
                                         func=Act.Copy)
                    nc.sync.dma_start(out=d_out[half * 2 + q], in_=oq[:])

    nc.compile()
    return nc


def _prep_core_inputs(i, features, surface_normal, valid_f, taps, variants):
    """Host-side shard prep for core i -> dict of device arrays.

    Builds one contiguous [128, ...] image per (dy, parity) variant so each
    device load is a single dense DMA whose outer dim (128) sprays across
    all 16 DMA engines. Padded row yext <-> image row r0 - 4 + yext; padded
    col jj <-> image col jj - 4 (pad 4 left so every variant window is
    in-bounds).
    """
    b = i // 4
    r0 = (i % 4) * RCH
    lo = max(0, r0 - 4)
    hi = min(H, r0 + RCH + 4)
    ylo = lo - (r0 - 4)
    yhi = hi - (r0 - 4)

    fp = np.zeros((YEXT, C, PW), BF16)
    fp[ylo:yhi, :, 4:4 + W] = features[b, :, lo:hi, :].transpose(1, 0, 2)
    npd = np.zeros((YEXT, 3, PW), BF16)
    npd[ylo:yhi, :, 4:4 + W] = surface_normal[b, :, lo:hi, :].transpose(1, 0, 2)
    vp = np.zeros((YEXT, PW), BF16)
    vp[ylo:yhi, 4:4 + W] = valid_f[b, lo:hi, :]

    sn_view = surface_normal.reshape(B, H, W, 3)  # raw memory reinterpret
    clo = max(0, r0 - 2)
    chi = min(H, r0 + RCH + 2)
    nre_rows = np.zeros((ROWS, W, 3), np.float32)
    nre_rows[clo - (r0 - 2):chi - (r0 - 2)] = sn_view[b, clo:chi]
    nre = np.ascontiguousarray(np.concatenate(
        [nre_rows[:, xh * XH:(xh + 1) * XH, :].transpose(0, 2, 1)
         for xh in (0, 1)], 0)).astype(BF16)

    # feat variant (dy, par): tile[xh*64+y, ..., jj] = img[y + dy + 2, ...,
    # xh*XH + jj + par - 2] -> padded col offset xh*XH + par + 2.
    NV = len(variants)
    feat = np.empty((NV, 128, C, XW), BF16)
    for vi, (dy, par) in enumerate(variants):
        ys = dy + 2
        for xh in (0, 1):
            xs = xh * XH + par + 2
            feat[vi, xh * ROWS:(xh + 1) * ROWS] = \
                fp[ys:ys + ROWS, :, xs:xs + XW]
    # weight-pipeline inputs: fully-shifted per-tap images
    U = len(taps)
    nrm = np.empty((128, U, 3, XH), BF16)
    vld = np.empty((128, U, XH), BF16)
    for u, ((dy, dx), m) in enumerate(taps):
        ys = dy + 2
        for xh in (0, 1):
            xs = xh * XH + dx + 4
            nrm[xh * ROWS:(xh + 1) * ROWS, u] = \
                npd[ys:ys + ROWS, :, xs:xs + XH]
            vld[xh * ROWS:(xh + 1) * ROWS, u] = vp[ys:ys + ROWS, xs:xs + XH]
    return {"feat": feat, "nrm": nrm, "vld": vld, "nre": nre}


def _run_device(inputs, trace=False):
    features = np.ascontiguousarray(np.asarray(inputs["features"], np.float32))
    surface_normal = np.ascontiguousarray(
        np.asarray(inputs["surface_normal"], np.float32))
    depth = np.asarray(inputs["depth"], np.float32)
    sample_idx = np.asarray(inputs["sample_idx"])

    d = depth[:, 0]
    valid_f = ((d > 0) & (d < DEPTH_MAX)).astype(np.float32)

    taps = _unique_taps(sample_idx)
    if taps not in _compiled:
        _compiled[taps] = _build(taps)
    nc = _compiled[taps]

    variants = _variants(taps)
    mults = sorted({m for _, m in taps})
    idm = np.stack([np.eye(128, dtype=np.float32) * m
                    for m in mults]).astype(BF16)
    in_maps = []
    for i in range(NCORES):
        m = _prep_core_inputs(i, features, surface_normal, valid_f, taps,
                              variants)
        m["idm"] = idm
        in_maps.append(m)
    res = run_bass_kernel_spmd(nc, in_maps, list(range(NCORES)), trace=trace)

    out = np.empty((B, C, H, W), np.float32)
    for i in range(NCORES):
        b = i // 4
        r0 = (i % 4) * RCH
        o = np.asarray(res.results[i]["out"], np.float32)  # [4,128,C,QS]
        QS = XH // 4
        for q in range(4):
            for xh in (0, 1):
                sl = o[q, xh * ROWS + 2: xh * ROWS + 2 + RCH]  # [RCH, C, QS]
                out[b, :, r0:r0 + RCH,
                    xh * XH + q * QS: xh * XH + (q + 1) * QS] = \
                    sl.transpose(1, 0, 2)
    return out, res


def _reference_numpy(depth, surface_normal, features, guide_weight, sample_idx):
    """Plain-numpy port of the reference (general fallback path)."""
    b, c, h, w = features.shape
    d = depth[:, 0]
    valid = ((d > 0) & (d < DEPTH_MAX)).astype(features.dtype)[:, None]

    def gather(x):
        B_, C_, H_, W_ = x.shape
        xp = np.pad(x, ((0, 0), (0, 0), (2, 2), (2, 2)))
        slabs = []
        for i in range(SAMPLE_NUM):
            p = int(sample_idx[i])
            dy, dx = p // K_SIZE, p % K_SIZE
            slabs.append(xp[:, :, dy:dy + H_, dx:dx + W_])
        return np.stack(slabs, 1).transpose(0, 3, 4, 1, 2)  # [B,H,W,S,C]

    feat_s = gather(features)
    norm_s = gather(surface_normal)
    valid_s = gather(valid)[..., 0]
    center_n = surface_normal.reshape(b, h, w, 3)
    diff = np.sqrt(((norm_s - center_n[:, :, :, None, :]) ** 2).sum(-1))
    normal_w = np.exp(-0.5 * diff)
    guide_s = guide_weight[..., np.asarray(sample_idx)]
    fw = valid_s * normal_w * guide_s
    fw = fw - fw.max(-1, keepdims=True)
    fw = np.exp(fw)
    fw = fw / fw.sum(-1, keepdims=True)
    out = (feat_s * fw[..., None]).sum(3)
    return out.transpose(0, 3, 1, 2).astype(features.dtype)


def kernel(**inputs):
    features = np.asarray(inputs["features"])
    guide = np.asarray(inputs["guide_weight"])
    if not np.all(guide == 1.0):
        # General path (never taken for this problem's spec: fill=ones).
        out = _reference_numpy(
            np.asarray(inputs["depth"], np.float32),
            np.ascontiguousarray(np.asarray(inputs["surface_normal"], np.float32)),
            np.ascontiguousarray(np.asarray(inputs["features"], np.float32)),
            np.asarray(guide, np.float32),
            np.asarray(inputs["sample_idx"]))
        return out, features
    out, _ = _run_device(inputs)
    return out, features


if __name__ == "__main__":
    rng = np.random.default_rng(0)
    inputs = {
        "depth": rng.uniform(0, 200, (B, 1, H, W)).astype(np.float32),
        "surface_normal": rng.standard_normal((B, 3, H, W)).astype(np.float32),
        "features": rng.standard_normal((B, C, H, W)).astype(np.float32),
        "guide_weight": np.ones((B, H, W, 25), np.float32),
        "sample_idx": rng.integers(0, 25, 15).astype(np.int32),
    }
    out, _ = kernel(**inputs)
    exp = _reference_numpy(
        inputs["depth"], inputs["surface_normal"], inputs["features"],
        inputs["guide_weight"], inputs["sample_idx"])
    err = np.linalg.norm(out - exp) / np.linalg.norm(exp)
    print("smoke rel err:", err)
